# revision 20
# baseline (speedup 1.0000x reference)
"""ExtractTensorPatches kernel for 8 trn2 NeuronCores.

Problem: x (4, 32, 256, 256) f32 -> out (4, 961, 32, 16, 16) f32 with
  out[b, ho*31+wo, c, i, j] = x[b, c, 8*ho+i, 8*wo+j] + EPS * patchsum
  patchsum = sum over the 16x16 patch at (8*ho, 8*wo).

Sharding: pure data parallelism over channels. Core k handles channels
[4k, 4k+4) for all 4 batches. Host gathers with concat on axis 2.

Per-core layout (one tile set per batch b):
  X tile  [124, 4096]: partition p=(hp, c) (hp=band 0..30, c=0..3); the
          partition holds rows 8*hp..8*hp+15 (16 rows x 256 cols) of
          channel c. Adjacent bands overlap by 8 rows -> 2x read amp,
          but every DMA run is 4KB+ contiguous DRAM.
  R1      [124, 512]: per-(row i, 8-col-block k) partial sums.
  S       [124, 31]:  per-(band, wo) 16x16 patch sums.
  OUT     [124, 7936]: free = (wo, i, j); computed in ONE fused DVE op
          out = (S * EPS) + X_widened, where X is read with the
          overlapping window AP (wo stride 8, window 16).
  OUT DMA: partition (hp, c) -> out[b, hp*31+wo, c, i, j]; 1KB runs.
"""

import sys

for _p in ("/opt/trn_rl_repo", "/root/.axon_site/_ro/trn_rl_repo"):
    if _p not in sys.path:
        sys.path.append(_p)

import numpy as np

B, C, H, W = 4, 32, 256, 256
WIN, STR = 16, 8
HO = (H - WIN) // STR + 1  # 31
L = HO * HO  # 961
EPS = 1e-6
NCORES = 8
CLOC = C // NCORES  # 4 channels per core
NP_PART = HO * CLOC  # 124 partitions in use

_nc_cache = {}


def _mk(t, dims):
    """Build a custom AP on a pool tile: partition dim + given free dims."""
    import concourse.bass as bass

    pstep = 1
    for d in t.tensor.shape[1:]:
        pstep *= d
    return bass.AP(t.tensor, t.offset, [[pstep, t.shape[0]]] + [list(d) for d in dims])


def build_nc():
    import concourse.bacc as bacc
    import concourse.mybir as mybir
    import concourse.tile as tile

    f32 = mybir.dt.float32
    nc = bacc.Bacc(
        "TRN2", target_bir_lowering=False, debug=False, num_devices=NCORES
    )
    x = nc.dram_tensor("x", [B, CLOC, H, W], f32, kind="ExternalInput").ap()
    # per-core layout (B, C_loc, ho, wo, i, j): each SBUF partition's
    # store is one fully-contiguous 31744B DRAM chunk (host permutes
    # back to (B, L, C, i, j) during the unshard gather).
    out = nc.dram_tensor(
        "out", [B, CLOC, HO, HO, WIN, WIN], f32, kind="ExternalOutput"
    ).ap()
    import concourse.bass as bass

    # SWDGE round-robin engine pointer: each gpsimd dma_start lands fully
    # on the next SDMA engine (mod 16). Loads run on HWDGE engines 0-3
    # (partition//32), so steer stores onto engines 4-15 with tiny dummy
    # DMAs that burn pointer slots 0-3.
    swdge_ptr = [0]
    dummy_dram = nc.dram_tensor("rr_align", [16, 1], f32).ap()

    with tile.TileContext(nc) as tc:
        with (
            tc.tile_pool(name="xin", bufs=3) as xpool,
            tc.tile_pool(name="stats", bufs=2) as spool,
            tc.tile_pool(name="outp", bufs=3) as opool,
        ):

            def align_store_group(OUT, lo, hi):
                while swdge_ptr[0] % 16 < 4:
                    k = swdge_ptr[0] % 16
                    nc.gpsimd.dma_start(
                        out=dummy_dram[k : k + 1, :], in_=OUT[0:1, lo : lo + 1]
                    )
                    swdge_ptr[0] += 1

            for b in range(B):
                # ---- load: partition (c, hp) <- rows 8hp..8hp+15 of chan c
                X = xpool.tile([NP_PART, WIN * W], f32, tag="X")
                src = bass.AP(
                    x.tensor,
                    b * CLOC * H * W,
                    [[H * W, CLOC], [STR * W, HO], [1, WIN * W]],
                )
                nc.sync.dma_start(out=_mk(X, [[1, WIN * W]]), in_=src)

                # ---- R1[p, i*32+k] = sum_{j8} X[p, i*256 + 8k + j8]
                R1 = spool.tile([NP_PART, WIN * 32], f32, tag="R1")
                nc.vector.reduce_sum(
                    out=_mk(R1, [[1, WIN * 32]]),
                    in_=_mk(X, [[W, WIN], [8, 32], [1, 8]]),
                    axis=mybir.AxisListType.X,
                )
                # ---- S[p, wo] = sum_{i, d in {0,1}} R1[p, i*32 + wo + d]
                S = spool.tile([NP_PART, HO], f32, tag="S")
                nc.vector.reduce_sum(
                    out=_mk(S, [[1, HO]]),
                    in_=_mk(R1, [[1, HO], [32, WIN], [1, 2]]),
                    axis=mybir.AxisListType.XY,
                )

                # ---- OUT[p, (wo,i,j)] = (S[p,wo] * EPS) + X[p, i*256+8wo+j]
                # walrus requires <=3D stt inputs -> one op per patch row i;
                # split by wo-halves so each half's stores launch early.
                OUT = opool.tile([NP_PART, HO * WIN * WIN], f32, tag="OUT")
                opstep = 1
                for d in OUT.tensor.shape[1:]:
                    opstep *= d
                xpstep = 1
                for d in X.tensor.shape[1:]:
                    xpstep *= d
                fpp = HO * WIN * WIN  # 7936 elems per partition
                for wlo, whi in ((0, 16), (16, HO)):
                    nwo = whi - wlo
                    for i in range(WIN):
                        out_ap = bass.AP(
                            OUT.tensor,
                            OUT.offset + wlo * WIN * WIN + i * WIN,
                            [[opstep, NP_PART], [WIN * WIN, nwo], [1, WIN]],
                        )
                        in1_ap = bass.AP(
                            X.tensor,
                            X.offset + wlo * STR + i * W,
                            [[xpstep, NP_PART], [STR, nwo], [1, WIN]],
                        )
                        nc.vector.scalar_tensor_tensor(
                            out=out_ap,
                            in0=bass.AP(
                                S.tensor,
                                S.offset + wlo,
                                [[S.tensor.shape[1], NP_PART], [1, nwo], [0, WIN]],
                            ),
                            scalar=float(EPS),
                            in1=in1_ap,
                            op0=mybir.AluOpType.mult,
                            op1=mybir.AluOpType.add,
                        )

                    # ---- store half: partition (c, hp) -> out[b,c,hp,wlo:whi]
                    # Late second halves ride the otherwise-idle scalar
                    # HWDGE ring (engines 0-3 are free once loads finish).
                    use_scalar = b >= 2 and wlo == 16
                    if not use_scalar:
                        align_store_group(OUT, wlo * WIN * WIN, whi * WIN * WIN)
                    for c in range(CLOC):
                        dst = bass.AP(
                            out.tensor,
                            (b * CLOC + c) * HO * fpp + wlo * WIN * WIN,
                            [[fpp, HO], [1, nwo * WIN * WIN]],
                        )
                        src_sb = OUT[
                            c * HO : (c + 1) * HO,
                            wlo * WIN * WIN : whi * WIN * WIN,
                        ]
                        if use_scalar:
                            nc.scalar.dma_start(out=dst, in_=src_sb)
                        else:
                            nc.gpsimd.dma_start(out=dst, in_=src_sb)
                            swdge_ptr[0] += 1

    nc.compile()
    return nc


def get_nc():
    if "nc" not in _nc_cache:
        _nc_cache["nc"] = build_nc()
    return _nc_cache["nc"]


def kernel(x: np.ndarray) -> np.ndarray:
    from concourse.bass_utils import run_bass_kernel_spmd

    x = np.ascontiguousarray(np.asarray(x, dtype=np.float32))
    nc = get_nc()
    in_maps = [
        {"x": np.ascontiguousarray(x[:, k * CLOC : (k + 1) * CLOC])}
        for k in range(NCORES)
    ]
    res = run_bass_kernel_spmd(nc, in_maps, list(range(NCORES)))
    # res[k]["out"]: (B, CLOC, ho, wo, i, j) -> full (B, L, C, i, j)
    arr = np.stack([r["out"] for r in res.results], axis=0)
    return np.ascontiguousarray(
        arr.transpose(1, 3, 4, 0, 2, 5, 6).reshape(B, L, C, WIN, WIN)
    )


# revision 21
# speedup vs baseline: 1.3227x; 1.3227x over previous
"""ExtractTensorPatches kernel for 8 trn2 NeuronCores.

Problem: x (4, 32, 256, 256) f32 -> out (4, 961, 32, 16, 16) f32 with
  out[b, ho*31+wo, c, i, j] = x[b, c, 8*ho+i, 8*wo+j] + EPS * patchsum
  patchsum = sum over the 16x16 patch at (8*ho, 8*wo).

Sharding: pure data parallelism over channels. Core k handles channels
[4k, 4k+4) for all 4 batches. Host gathers with concat on axis 2.

Per-core layout (one tile set per batch b):
  X tile  [124, 4096]: partition p=(hp, c) (hp=band 0..30, c=0..3); the
          partition holds rows 8*hp..8*hp+15 (16 rows x 256 cols) of
          channel c. Adjacent bands overlap by 8 rows -> 2x read amp,
          but every DMA run is 4KB+ contiguous DRAM.
  R1      [124, 512]: per-(row i, 8-col-block k) partial sums.
  S       [124, 31]:  per-(band, wo) 16x16 patch sums.
  OUT     [124, 7936]: free = (wo, i, j); computed in ONE fused DVE op
          out = (S * EPS) + X_widened, where X is read with the
          overlapping window AP (wo stride 8, window 16).
  OUT DMA: partition (hp, c) -> out[b, hp*31+wo, c, i, j]; 1KB runs.
"""

import sys

for _p in ("/opt/trn_rl_repo", "/root/.axon_site/_ro/trn_rl_repo"):
    if _p not in sys.path:
        sys.path.append(_p)

import numpy as np

B, C, H, W = 4, 32, 256, 256
WIN, STR = 16, 8
HO = (H - WIN) // STR + 1  # 31
L = HO * HO  # 961
EPS = 1e-6
NCORES = 8
CLOC = C // NCORES  # 4 channels per core
NP_PART = HO * CLOC  # 124 partitions in use

_nc_cache = {}


def _mk(t, dims):
    """Build a custom AP on a pool tile: partition dim + given free dims."""
    import concourse.bass as bass

    pstep = 1
    for d in t.tensor.shape[1:]:
        pstep *= d
    return bass.AP(t.tensor, t.offset, [[pstep, t.shape[0]]] + [list(d) for d in dims])


def build_nc():
    import concourse.bacc as bacc
    import concourse.mybir as mybir
    import concourse.tile as tile

    f32 = mybir.dt.float32
    nc = bacc.Bacc(
        "TRN2", target_bir_lowering=False, debug=False, num_devices=NCORES
    )
    x = nc.dram_tensor("x", [B, CLOC, H, W], f32, kind="ExternalInput").ap()
    # per-core layout (B, C_loc, ho, wo, i, j): each SBUF partition's
    # store is one fully-contiguous 31744B DRAM chunk (host permutes
    # back to (B, L, C, i, j) during the unshard gather).
    out = nc.dram_tensor(
        "out", [B, CLOC, HO, HO, WIN, WIN], f32, kind="ExternalOutput"
    ).ap()
    import concourse.bass as bass

    # SWDGE round-robin engine pointer: each gpsimd dma_start lands fully
    # on the next SDMA engine (mod 16). Loads run on HWDGE engines 0-3
    # (partition//32), so steer stores onto engines 4-15 with tiny dummy
    # DMAs that burn pointer slots 0-3.
    swdge_ptr = [0]
    dummy_dram = nc.dram_tensor("rr_align", [16, 1], f32).ap()

    with tile.TileContext(nc) as tc:
        with (
            tc.tile_pool(name="xin", bufs=3) as xpool,
            tc.tile_pool(name="stats", bufs=2) as spool,
            tc.tile_pool(name="outp", bufs=3) as opool,
        ):

            def align_store_group(OUT, lo, hi):
                while swdge_ptr[0] % 16 < 4:
                    k = swdge_ptr[0] % 16
                    nc.gpsimd.dma_start(
                        out=dummy_dram[k : k + 1, :], in_=OUT[0:1, lo : lo + 1]
                    )
                    swdge_ptr[0] += 1

            for b in range(B):
                # ---- load: partition (c, hp) <- rows 8hp..8hp+15 of chan c
                X = xpool.tile([NP_PART, WIN * W], f32, tag="X")
                src = bass.AP(
                    x.tensor,
                    b * CLOC * H * W,
                    [[H * W, CLOC], [STR * W, HO], [1, WIN * W]],
                )
                nc.sync.dma_start(out=_mk(X, [[1, WIN * W]]), in_=src)

                # ---- R1[p, i*32+k] = sum_{j8} X[p, i*256 + 8k + j8]
                R1 = spool.tile([NP_PART, WIN * 32], f32, tag="R1")
                nc.vector.reduce_sum(
                    out=_mk(R1, [[1, WIN * 32]]),
                    in_=_mk(X, [[W, WIN], [8, 32], [1, 8]]),
                    axis=mybir.AxisListType.X,
                )
                # ---- S[p, wo] = sum_{i, d in {0,1}} R1[p, i*32 + wo + d]
                S = spool.tile([NP_PART, HO], f32, tag="S")
                nc.vector.reduce_sum(
                    out=_mk(S, [[1, HO]]),
                    in_=_mk(R1, [[1, HO], [32, WIN], [1, 2]]),
                    axis=mybir.AxisListType.XY,
                )

                # ---- OUT[p, (wo,i,j)] = (S[p,wo] * EPS) + X[p, i*256+8wo+j]
                # walrus requires <=3D stt inputs -> one op per patch row i;
                # split by wo-halves so each half's stores launch early.
                OUT = opool.tile([NP_PART, HO * WIN * WIN], f32, tag="OUT")
                opstep = 1
                for d in OUT.tensor.shape[1:]:
                    opstep *= d
                xpstep = 1
                for d in X.tensor.shape[1:]:
                    xpstep *= d
                fpp = HO * WIN * WIN  # 7936 elems per partition
                for wlo, whi in ((0, 16), (16, HO)):
                    nwo = whi - wlo
                    for i in range(WIN):
                        out_ap = bass.AP(
                            OUT.tensor,
                            OUT.offset + wlo * WIN * WIN + i * WIN,
                            [[opstep, NP_PART], [WIN * WIN, nwo], [1, WIN]],
                        )
                        in1_ap = bass.AP(
                            X.tensor,
                            X.offset + wlo * STR + i * W,
                            [[xpstep, NP_PART], [STR, nwo], [1, WIN]],
                        )
                        nc.vector.scalar_tensor_tensor(
                            out=out_ap,
                            in0=bass.AP(
                                S.tensor,
                                S.offset + wlo,
                                [[S.tensor.shape[1], NP_PART], [1, nwo], [0, WIN]],
                            ),
                            scalar=float(EPS),
                            in1=in1_ap,
                            op0=mybir.AluOpType.mult,
                            op1=mybir.AluOpType.add,
                        )

                    # ---- store half: partition (c, hp) -> out[b,c,hp,wlo:whi]
                    align_store_group(OUT, wlo * WIN * WIN, whi * WIN * WIN)
                    for c in range(CLOC):
                        dst = bass.AP(
                            out.tensor,
                            (b * CLOC + c) * HO * fpp + wlo * WIN * WIN,
                            [[fpp, HO], [1, nwo * WIN * WIN]],
                        )
                        nc.gpsimd.dma_start(
                            out=dst,
                            in_=OUT[
                                c * HO : (c + 1) * HO,
                                wlo * WIN * WIN : whi * WIN * WIN,
                            ],
                        )
                        swdge_ptr[0] += 1

    nc.compile()
    return nc


def get_nc():
    if "nc" not in _nc_cache:
        _nc_cache["nc"] = build_nc()
    return _nc_cache["nc"]


def kernel(x: np.ndarray) -> np.ndarray:
    from concourse.bass_utils import run_bass_kernel_spmd

    x = np.ascontiguousarray(np.asarray(x, dtype=np.float32))
    nc = get_nc()
    in_maps = [
        {"x": np.ascontiguousarray(x[:, k * CLOC : (k + 1) * CLOC])}
        for k in range(NCORES)
    ]
    res = run_bass_kernel_spmd(nc, in_maps, list(range(NCORES)))
    # res[k]["out"]: (B, CLOC, ho, wo, i, j) -> full (B, L, C, i, j)
    arr = np.stack([r["out"] for r in res.results], axis=0)
    return np.ascontiguousarray(
        arr.transpose(1, 3, 4, 0, 2, 5, 6).reshape(B, L, C, WIN, WIN)
    )
